# Initial kernel scaffold
#
"""2-layer GCN on 8 TRN2 NeuronCores via Bass/Tile.

Math (per reference):
  deg[n]  = |{e : dst==n}| + 1   (self loops), dinv = deg^-1/2
  u1      = dinv[:,None] * (X @ W1)
  agg1[d] = sum_{e: dst=d} u1[src_e]      (incl. self loop)
  x1      = relu(dinv*agg1 + b1)
  v       = (dinv[:,None] * x1) @ W2
  agg2[d] = sum_{e: dst=d} v[src_e]
  out[d]  = dinv[d]*agg2[d] + b2

Distribution: nodes sharded contiguously across 8 cores (by dst).
Each core computes u1/v for its shard; AllGather makes the full
gather tables; per-core edge streams (grouped by chunk-of-blocks x
src-range x dst-block, padded to 128-edge tiles) drive dma_gather +
one-hot segment matmuls accumulating in PSUM.
"""
import math
import numpy as np

import concourse.bacc as bacc
import concourse.mybir as mybir
import concourse.tile as tile
from concourse.masks import make_identity

P = 128
F32 = mybir.dt.float32
I16 = mybir.dt.int16


def plan_graph(N, C, src, dst, *, range_cap=23334, blocks_per_chunk=16):
    """Group edges (with self loops) per core into (chunk, range, block)
    tiles of 128, padded uniformly across cores (SPMD needs one shape).

    Returns dict with per-core streams and the uniform tile table.
    """
    S = N // C
    assert S * C == N
    B = math.ceil(S / P)
    n_ranges = math.ceil(N / range_cap)
    range_bounds = [min(i * range_cap, N) for i in range(n_ranges + 1)]
    n_chunks = math.ceil(B / blocks_per_chunk)

    loops = np.arange(N, dtype=np.int64)
    s = np.concatenate([np.asarray(src), loops])
    d = np.concatenate([np.asarray(dst), loops])

    deg = np.bincount(d, minlength=N).astype(np.float32)  # includes +1 self loop

    core = d // S
    local = d - core * S
    blk = local // P
    dstloc = (local % P).astype(np.float32)
    rng_id = np.minimum(s // range_cap, n_ranges - 1)
    chunk = blk // blocks_per_chunk

    # group key in stream order: (core, chunk, rng, blk)
    key = ((core * n_chunks + chunk) * n_ranges + rng_id) * B + blk
    order = np.argsort(key, kind="stable")
    s_o, key_o, dstloc_o, rng_o = s[order], key[order], dstloc[order], rng_id[order]

    n_keys = C * n_chunks * n_ranges * B
    sizes = np.bincount(key_o, minlength=n_keys).reshape(C, n_chunks, n_ranges, B)
    # uniform tiles per (chunk, rng, blk): max over cores, >= 1
    tiles = np.maximum(np.ceil(sizes / P).astype(np.int64).max(axis=0), 1)
    # zero out blocks that don't exist in the tail chunk
    for ch in range(n_chunks):
        for b in range(B):
            if b // blocks_per_chunk != ch:
                tiles[ch, :, b] = 0
    T_total = int(tiles.sum())
    L = T_total * P

    starts = np.zeros(n_keys + 1, dtype=np.int64)
    starts[1:] = np.cumsum(sizes.reshape(-1))

    idx_streams, dl_streams = [], []
    for c in range(C):
        idx_st = np.zeros(L, dtype=np.int16)
        dl_st = np.full(L, -1.0, dtype=np.float32)
        off = 0
        for ch in range(n_chunks):
            for r in range(n_ranges):
                for b in range(B):
                    t = int(tiles[ch, r, b])
                    if t == 0:
                        continue
                    k = ((c * n_chunks + ch) * n_ranges + r) * B + b
                    a, e = starts[k], starts[k + 1]
                    n = e - a
                    idx_st[off:off + n] = (s_o[a:e] - range_bounds[r]).astype(np.int16)
                    dl_st[off:off + n] = dstloc_o[a:e]
                    off += t * P
        assert off == L
        idx_streams.append(np.tile(idx_st.reshape(-1, 16).T, (8, 1)))  # [128, L/16]
        dl_streams.append(np.ascontiguousarray(dl_st.reshape(T_total, P).T))  # [128, T]

    return dict(
        S=S, B=B, n_ranges=n_ranges, range_bounds=range_bounds,
        n_chunks=n_chunks, blocks_per_chunk=blocks_per_chunk,
        tiles=tiles, T_total=T_total, deg=deg,
        idx_streams=idx_streams, dl_streams=dl_streams,
    )


def build_nc(plan, N, C, IN_CH, F, OUT_CH, *, VPAD=64, G=32, MM_DT=F32, debug=False):
    """Build the Bass module. Returns finalized nc."""
    S, B = plan["S"], plan["B"]
    n_ranges, n_chunks = plan["n_ranges"], plan["n_chunks"]
    range_bounds = plan["range_bounds"]
    BC = plan["blocks_per_chunk"]
    tiles = plan["tiles"]
    T_total = plan["T_total"]
    L = T_total * P
    KT = math.ceil(IN_CH / P)
    NW = math.ceil(S / 512)

    nc = bacc.Bacc("TRN2", num_devices=C)

    xt_d = nc.dram_tensor("xt", [IN_CH, S], F32, kind="ExternalInput")
    w1_d = nc.dram_tensor("w1", [IN_CH, F], F32, kind="ExternalInput")
    w2_d = nc.dram_tensor("w2", [F, OUT_CH], F32, kind="ExternalInput")
    b1_d = nc.dram_tensor("b1r", [1, F], F32, kind="ExternalInput")
    b2_d = nc.dram_tensor("b2r", [1, OUT_CH], F32, kind="ExternalInput")
    degc_d = nc.dram_tensor("degc", [P, B], F32, kind="ExternalInput")
    degr_d = nc.dram_tensor("degr", [1, B * P], F32, kind="ExternalInput")
    idx_d = nc.dram_tensor("idx", [P, L // 16], I16, kind="ExternalInput")
    dl_d = nc.dram_tensor("dl", [P, T_total], F32, kind="ExternalInput")
    out_d = nc.dram_tensor("out", [B * P, OUT_CH], F32, kind="ExternalOutput")

    if debug:
        dbg_u1 = nc.dram_tensor("dbg_u1", [N, F], F32, kind="ExternalOutput")
        dbg_v = nc.dram_tensor("dbg_v", [N, VPAD], F32, kind="ExternalOutput")
        dbg_agg1 = nc.dram_tensor("dbg_agg1", [B * P, F], F32, kind="ExternalOutput")
    u1_shard = nc.dram_tensor("u1_shard", [S, F], F32)
    u1_full = nc.dram_tensor("u1_full", [N, F], F32, addr_space="Shared")
    v_shard = nc.dram_tensor("v_shard", [S, VPAD], F32)
    v_full = nc.dram_tensor("v_full", [N, VPAD], F32, addr_space="Shared")

    rg = [list(range(C))]

    with tile.TileContext(nc) as tc:
        cst = tc.alloc_tile_pool(name="const", bufs=1)
        iota = cst.tile([P, P], F32, name="iota")
        nc.gpsimd.iota(iota[:], pattern=[[1, P]], base=0, channel_multiplier=0,
                       allow_small_or_imprecise_dtypes=True)
        ident = cst.tile([P, P], F32, name="ident")
        make_identity(nc, ident[:])
        w2_sb = cst.tile([F, OUT_CH], F32, name="w2_sb")
        nc.sync.dma_start(out=w2_sb[:], in_=w2_d[:])
        b1_sb = cst.tile([1, F], F32, name="b1_sb")
        nc.sync.dma_start(out=b1_sb[:], in_=b1_d[:])
        b2_sb = cst.tile([1, OUT_CH], F32, name="b2_sb")
        nc.sync.dma_start(out=b2_sb[:], in_=b2_d[:])
        w1_sb = cst.tile([P, KT * F], F32, name="w1_sb")  # kt tile at [:, kt*F:]
        for kt in range(KT):
            kk = min(P, IN_CH - kt * P)
            nc.sync.dma_start(out=w1_sb[:kk, kt * F:(kt + 1) * F],
                              in_=w1_d[kt * P:kt * P + kk, :])
        degc_sb = cst.tile([P, B], F32, name="degc_sb")
        nc.sync.dma_start(out=degc_sb[:], in_=degc_d[:])
        dinv_sb = cst.tile([P, B], F32, name="dinv_sb")
        nc.vector.reciprocal(out=dinv_sb[:], in_=degc_sb[:])
        nc.scalar.activation(out=dinv_sb[:], in_=dinv_sb[:],
                             func=mybir.ActivationFunctionType.Sqrt)
        degr_sb = cst.tile([1, B * P], F32, name="degr_sb")
        nc.sync.dma_start(out=degr_sb[:], in_=degr_d[:])
        sqd_sb = cst.tile([1, B * P], F32, name="sqd_sb")
        nc.scalar.activation(out=sqd_sb[:], in_=degr_sb[:],
                             func=mybir.ActivationFunctionType.Sqrt)
        dl_sb = cst.tile([P, T_total], F32, name="dl_sb")
        nc.sync.dma_start(out=dl_sb[:], in_=dl_d[:])
        vslab = cst.tile([P, B * VPAD], F32, name="vslab")
        nc.gpsimd.memset(vslab[:], 0.0)
        o2slab = cst.tile([P, B * OUT_CH], F32, name="o2slab")

        # ---- Phase A: u1 = dinv * (X @ W1), via hT windows + PE transpose ----
        with (
            tc.tile_pool(name="pa_sb", bufs=3) as pa,
            tc.tile_pool(name="pa_ps", bufs=2, space="PSUM") as pap,
            tc.tile_pool(name="pa_ps_t", bufs=2, space="PSUM") as papt,
        ):
            for w in range(NW):
                cols = min(512, S - w * 512)
                hT_ps = pap.tile([P, 512], F32, tag="hT_ps", name="hT_ps")
                for kt in range(KT):
                    kk = min(P, IN_CH - kt * P)
                    xt_sb = pa.tile([P, 512], F32, tag="xt", name="xt_sb")
                    nc.sync.dma_start(
                        out=xt_sb[:kk, :cols],
                        in_=xt_d[kt * P:kt * P + kk, w * 512:w * 512 + cols])
                    nc.tensor.matmul(out=hT_ps[:, :cols],
                                     lhsT=w1_sb[:kk, kt * F:(kt + 1) * F],
                                     rhs=xt_sb[:kk, :cols],
                                     start=(kt == 0), stop=(kt == KT - 1))
                hT_sb = pa.tile([P, 512], F32, tag="hT_sb", name="hT_sb")
                nc.vector.tensor_copy(out=hT_sb[:, :cols], in_=hT_ps[:, :cols])
                for j in range(math.ceil(cols / P)):
                    cc = min(P, cols - j * P)
                    b = w * 4 + j
                    tps = papt.tile([P, P], F32, tag="tps", name="tps")
                    nc.tensor.transpose(out=tps[:cc, :], in_=hT_sb[:, j * P:j * P + cc],
                                        identity=ident[:])
                    u_sb = pa.tile([P, F], F32, tag="u_sb", name="u_sb")
                    nc.scalar.activation(out=u_sb[:cc, :], in_=tps[:cc, :],
                                         func=mybir.ActivationFunctionType.Copy,
                                         scale=dinv_sb[:cc, b:b + 1])
                    nc.sync.dma_start(out=u1_shard[b * P:b * P + cc, :], in_=u_sb[:cc, :])

        nc.gpsimd.collective_compute(
            "AllGather", mybir.AluOpType.bypass, replica_groups=rg,
            ins=[u1_shard[:]], outs=[u1_full[:]])
        if debug:
            nc.sync.dma_start(out=dbg_u1[:], in_=u1_full[:])

        # ---- Aggregation layers ----
        def agg_layer(layer):
            tab, elem = (u1_full, F) if layer == 0 else (v_full, VPAD)
            tile_ptr = 0
            with (
                tc.tile_pool(name=f"ag_sb{layer}", bufs=2) as ag,
                tc.tile_pool(name=f"ag_acc{layer}", bufs=BC, space="PSUM") as accp,
                tc.tile_pool(name=f"ag_epT{layer}", bufs=1, space="PSUM") as eppt,
                tc.tile_pool(name=f"ag_epV{layer}", bufs=1, space="PSUM") as eppv,
            ):
                for ch in range(n_chunks):
                    blocks = [b for b in range(ch * BC, min((ch + 1) * BC, B))]
                    accs = {b: accp.tile([P, elem], F32, tag="acc",
                                         name=f"acc_{layer}_{ch}_{b}")
                            for b in blocks}

                    def acc_ap(b):
                        return accs[b][:]

                    started = {b: False for b in blocks}
                    for r in range(n_ranges):
                        sec = []  # block id per tile, stream order
                        for b in blocks:
                            sec += [b] * int(tiles[ch, r, b])
                        base = range_bounds[r]
                        rlen = range_bounds[r + 1] - base
                        pos = 0
                        while pos < len(sec):
                            bt = sec[pos:pos + G]
                            gn = len(bt) * P
                            t0 = tile_ptr + pos
                            idx_sb = ag.tile([P, G * P // 16], I16, tag="idx", name="idx_sb")
                            nc.sync.dma_start(
                                out=idx_sb[:, :gn // 16],
                                in_=idx_d[:, (t0 * P) // 16:(t0 * P + gn) // 16])
                            gat = ag.tile([P, G, elem], F32, tag="gat", name="gat")
                            nc.gpsimd.dma_gather(
                                out_ap=gat[:, :len(bt), :],
                                in_ap=tab[base:base + rlen, :],
                                idxs_ap=idx_sb[:, :gn // 16],
                                num_idxs=gn, num_idxs_reg=gn, elem_size=elem,
                                single_packet=False)
                            st = ag.tile([P, G, P], MM_DT, tag="st", name="st")
                            nc.vector.tensor_tensor(
                                out=st[:, :len(bt), :],
                                in0=iota[:].rearrange("p (a f) -> p a f", a=1)
                                    .to_broadcast([P, len(bt), P]),
                                in1=dl_sb[:, t0:t0 + len(bt)]
                                    .rearrange("p (a f) -> p a f", f=1)
                                    .to_broadcast([P, len(bt), P]),
                                op=mybir.AluOpType.is_equal)
                            for i, b in enumerate(bt):
                                nc.tensor.matmul(
                                    out=acc_ap(b), lhsT=st[:, i, :], rhs=gat[:, i, :],
                                    start=(not started[b]), stop=False,
                                    skip_group_check=True)
                                started[b] = True
                            pos += len(bt)
                        tile_ptr += len(sec)

                    # epilogue per block
                    for b in blocks:
                        bias_rhs = b1_sb if layer == 0 else b2_sb
                        nw = F if layer == 0 else OUT_CH
                        nc.tensor.matmul(
                            out=acc_ap(b)[:, :nw],
                            lhsT=sqd_sb[:, b * P:(b + 1) * P],
                            rhs=bias_rhs[:],
                            start=False, stop=True, skip_group_check=True)
                        cc = min(P, S - b * P)
                        if layer == 0 and debug:
                            dbg_sb = ag.tile([P, F], F32, tag="dbg", name="dbg_sb")
                            nc.vector.tensor_copy(out=dbg_sb[:], in_=acc_ap(b))
                            nc.sync.dma_start(out=dbg_agg1[b * P:(b + 1) * P, :],
                                              in_=dbg_sb[:])
                        if layer == 0:
                            x1_sb = ag.tile([P, F], F32, tag="x1", name="x1_sb")
                            nc.scalar.activation(
                                out=x1_sb[:cc, :], in_=acc_ap(b)[:cc, :],
                                func=mybir.ActivationFunctionType.Relu,
                                scale=dinv_sb[:cc, b:b + 1])
                            u2_sb = ag.tile([P, F], F32, tag="u2", name="u2_sb")
                            nc.vector.tensor_scalar(
                                out=u2_sb[:cc, :], in0=x1_sb[:cc, :],
                                scalar1=dinv_sb[:cc, b:b + 1], scalar2=None,
                                op0=mybir.AluOpType.mult)
                            tps2 = epp.tile([P, P], F32, tag="tps2", name="tps2")
                            nc.tensor.transpose(out=tps2[:, :cc], in_=u2_sb[:cc, :],
                                                identity=ident[:cc, :cc])
                            u2T_sb = ag.tile([P, P], F32, tag="u2T", name="u2T_sb")
                            nc.scalar.copy(out=u2T_sb[:, :cc], in_=tps2[:, :cc])
                            v_ps = epp.tile([P, OUT_CH], F32, tag="v_ps", name="v_ps")
                            nc.tensor.matmul(out=v_ps[:cc, :], lhsT=u2T_sb[:, :cc],
                                             rhs=w2_sb[:], start=True, stop=True)
                            nc.vector.tensor_copy(
                                out=vslab[:cc, b * VPAD:b * VPAD + OUT_CH],
                                in_=v_ps[:cc, :])
                        else:
                            nc.scalar.activation(
                                out=o2slab[:cc, b * OUT_CH:(b + 1) * OUT_CH],
                                in_=acc_ap(b)[:cc, :OUT_CH],
                                func=mybir.ActivationFunctionType.Copy,
                                scale=dinv_sb[:cc, b:b + 1])

        agg_layer(0)
        # vslab -> v_shard  (blocks 0..B-2 full, last partial)
        nfull = (S // P)  # number of full blocks
        nc.sync.dma_start(
            out=v_shard[0:nfull * P, :].rearrange("(t p) f -> p t f", p=P),
            in_=vslab[:].rearrange("p (t f) -> p t f", f=VPAD)[:, :nfull, :])
        rem = S - nfull * P
        if rem:
            nc.sync.dma_start(
                out=v_shard[nfull * P:, :],
                in_=vslab[:rem, nfull * VPAD:(nfull + 1) * VPAD])
        nc.gpsimd.collective_compute(
            "AllGather", mybir.AluOpType.bypass, replica_groups=rg,
            ins=[v_shard[:]], outs=[v_full[:]])
        if debug:
            nc.sync.dma_start(out=dbg_v[:], in_=v_full[:])
        agg_layer(1)
        # o2slab -> out  [B*P, OUT_CH]
        nc.sync.dma_start(
            out=out_d[:].rearrange("(t p) f -> p t f", p=P),
            in_=o2slab[:].rearrange("p (t f) -> p t f", f=OUT_CH))
        cst.__exit__(None, None, None)

    nc.finalize()
    return nc


def make_inputs(plan, N, C, features, W1, b1, W2, b2):
    """Per-core input dicts (host-side sharding / transposes)."""
    S, B = plan["S"], plan["B"]
    deg = plan["deg"]
    ins = []
    for c in range(C):
        sl = slice(c * S, (c + 1) * S)
        deg_pad = np.ones(B * P, dtype=np.float32)
        deg_pad[:S] = deg[sl]
        ins.append({
            "xt": np.ascontiguousarray(features[sl].T),
            "w1": np.ascontiguousarray(W1),
            "w2": np.ascontiguousarray(W2),
            "b1r": np.ascontiguousarray(b1[None, :]),
            "b2r": np.ascontiguousarray(b2[None, :]),
            "degc": np.ascontiguousarray(deg_pad.reshape(B, P).T),
            "degr": deg_pad[None, :],
            "idx": plan["idx_streams"][c],
            "dl": plan["dl_streams"][c],
        })
    return ins


# ---------------------------------------------------------------------------
# Harness entry point: full inputs in, full output out.
# ---------------------------------------------------------------------------
N_NODES = 70000
N_CORES = 8
IN_CH_, FEAT_, OUT_ = 1044, 128, 3
RANGE_CAP = 23334        # int16 gather-index limit per table slice
BLOCKS_PER_CHUNK = 6     # live PSUM accumulators per chunk
GATHER_TILES = 32        # 128-edge tiles per dma_gather


def kernel(features, edges, edges2, edge_features, W1, b1, W2, b2):
    """2-layer GCN (PyG GCNConv x2, eval mode). edges2/edge_features unused
    by the module. Returns [70000, 3] float32."""
    from concourse.bass_utils import run_bass_kernel_spmd

    features = np.ascontiguousarray(np.asarray(features, dtype=np.float32))
    W1 = np.asarray(W1, dtype=np.float32)
    b1 = np.asarray(b1, dtype=np.float32)
    W2 = np.asarray(W2, dtype=np.float32)
    b2 = np.asarray(b2, dtype=np.float32)
    e = np.asarray(edges)
    src = e[0].astype(np.int64)
    dst = e[1].astype(np.int64)

    plan = plan_graph(N_NODES, N_CORES, src, dst, range_cap=RANGE_CAP,
                      blocks_per_chunk=BLOCKS_PER_CHUNK)
    nc = build_nc(plan, N_NODES, N_CORES, IN_CH_, FEAT_, OUT_, G=GATHER_TILES)
    ins = make_inputs(plan, N_NODES, N_CORES, features, W1, b1, W2, b2)
    res = run_bass_kernel_spmd(nc, ins, core_ids=list(range(N_CORES)))
    S = plan["S"]
    out = np.concatenate([res.results[c]["out"][:S] for c in range(N_CORES)],
                         axis=0)
    return out.astype(np.float32)


# revision 1
# speedup vs baseline: 619.8402x; 619.8402x over previous
"""2-layer GCN on 8 TRN2 NeuronCores via Bass/Tile.

Math (per reference):
  deg[n]  = |{e : dst==n}| + 1   (self loops), dinv = deg^-1/2
  u1      = dinv[:,None] * (X @ W1)
  agg1[d] = sum_{e: dst=d} u1[src_e]      (incl. self loop)
  x1      = relu(dinv*agg1 + b1)
  v       = (dinv[:,None] * x1) @ W2
  agg2[d] = sum_{e: dst=d} v[src_e]
  out[d]  = dinv[d]*agg2[d] + b2

Distribution: nodes sharded contiguously across 8 cores (by dst).
Each core computes u1/v for its shard; AllGather makes the full
gather tables; per-core edge streams (grouped by chunk-of-blocks x
src-range x dst-block, padded to 128-edge tiles) drive dma_gather +
one-hot segment matmuls accumulating in PSUM.
"""
import math
import numpy as np

import concourse.bacc as bacc
import concourse.mybir as mybir
import concourse.tile as tile
from concourse.masks import make_identity

P = 128
F32 = mybir.dt.float32
I16 = mybir.dt.int16


def plan_graph(N, C, src, dst, *, range_cap=23334, blocks_per_chunk=16):
    """Group edges (with self loops) per core into (chunk, range, block)
    tiles of 128, padded uniformly across cores (SPMD needs one shape).

    Returns dict with per-core streams and the uniform tile table.
    """
    S = N // C
    assert S * C == N
    B = math.ceil(S / P)
    n_ranges = math.ceil(N / range_cap)
    range_bounds = [min(i * range_cap, N) for i in range(n_ranges + 1)]
    n_chunks = math.ceil(B / blocks_per_chunk)

    loops = np.arange(N, dtype=np.int64)
    s = np.concatenate([np.asarray(src), loops])
    d = np.concatenate([np.asarray(dst), loops])

    deg = np.bincount(d, minlength=N).astype(np.float32)  # includes +1 self loop

    core = d // S
    local = d - core * S
    blk = local // P
    dstloc = (local % P).astype(np.float32)
    rng_id = np.minimum(s // range_cap, n_ranges - 1)
    chunk = blk // blocks_per_chunk

    # group key in stream order: (core, chunk, rng, blk)
    key = ((core * n_chunks + chunk) * n_ranges + rng_id) * B + blk
    order = np.argsort(key, kind="stable")
    s_o, key_o, dstloc_o, rng_o = s[order], key[order], dstloc[order], rng_id[order]

    n_keys = C * n_chunks * n_ranges * B
    sizes = np.bincount(key_o, minlength=n_keys).reshape(C, n_chunks, n_ranges, B)
    # uniform tiles per (chunk, rng, blk): max over cores, >= 1
    tiles = np.maximum(np.ceil(sizes / P).astype(np.int64).max(axis=0), 1)
    # zero out blocks that don't exist in the tail chunk
    for ch in range(n_chunks):
        for b in range(B):
            if b // blocks_per_chunk != ch:
                tiles[ch, :, b] = 0
    T_total = int(tiles.sum())
    L = T_total * P

    starts = np.zeros(n_keys + 1, dtype=np.int64)
    starts[1:] = np.cumsum(sizes.reshape(-1))

    idx_streams, dl_streams = [], []
    for c in range(C):
        idx_st = np.zeros(L, dtype=np.int16)
        dl_st = np.full(L, -1.0, dtype=np.float32)
        off = 0
        for ch in range(n_chunks):
            for r in range(n_ranges):
                for b in range(B):
                    t = int(tiles[ch, r, b])
                    if t == 0:
                        continue
                    k = ((c * n_chunks + ch) * n_ranges + r) * B + b
                    a, e = starts[k], starts[k + 1]
                    n = e - a
                    idx_st[off:off + n] = (s_o[a:e] - range_bounds[r]).astype(np.int16)
                    dl_st[off:off + n] = dstloc_o[a:e]
                    off += t * P
        assert off == L
        idx_streams.append(np.tile(idx_st.reshape(-1, 16).T, (8, 1)))  # [128, L/16]
        dl_streams.append(np.ascontiguousarray(dl_st.reshape(T_total, P).T))  # [128, T]

    return dict(
        S=S, B=B, n_ranges=n_ranges, range_bounds=range_bounds,
        n_chunks=n_chunks, blocks_per_chunk=blocks_per_chunk,
        tiles=tiles, T_total=T_total, deg=deg,
        idx_streams=idx_streams, dl_streams=dl_streams,
    )


def build_nc(plan, N, C, IN_CH, F, OUT_CH, *, VPAD=64, G=32, MM_DT=F32, debug=False):
    """Build the Bass module. Returns finalized nc."""
    S, B = plan["S"], plan["B"]
    n_ranges, n_chunks = plan["n_ranges"], plan["n_chunks"]
    range_bounds = plan["range_bounds"]
    BC = plan["blocks_per_chunk"]
    tiles = plan["tiles"]
    T_total = plan["T_total"]
    L = T_total * P
    KT = math.ceil(IN_CH / P)
    NW = math.ceil(S / 512)

    nc = bacc.Bacc("TRN2", num_devices=C)

    xt_d = nc.dram_tensor("xt", [IN_CH, S], F32, kind="ExternalInput")
    w1_d = nc.dram_tensor("w1", [IN_CH, F], F32, kind="ExternalInput")
    w2_d = nc.dram_tensor("w2", [F, OUT_CH], F32, kind="ExternalInput")
    b1_d = nc.dram_tensor("b1r", [1, F], F32, kind="ExternalInput")
    b2_d = nc.dram_tensor("b2r", [1, OUT_CH], F32, kind="ExternalInput")
    degc_d = nc.dram_tensor("degc", [P, B], F32, kind="ExternalInput")
    degr_d = nc.dram_tensor("degr", [1, B * P], F32, kind="ExternalInput")
    idx_d = nc.dram_tensor("idx", [P, L // 16], I16, kind="ExternalInput")
    dl_d = nc.dram_tensor("dl", [P, T_total], F32, kind="ExternalInput")
    out_d = nc.dram_tensor("out", [B * P, OUT_CH], F32, kind="ExternalOutput")

    if debug:
        dbg_u1 = nc.dram_tensor("dbg_u1", [N, F], F32, kind="ExternalOutput")
        dbg_v = nc.dram_tensor("dbg_v", [N, VPAD], F32, kind="ExternalOutput")
        dbg_agg1 = nc.dram_tensor("dbg_agg1", [B * P, F], F32, kind="ExternalOutput")
    u1_shard = nc.dram_tensor("u1_shard", [S, F], F32)
    u1_full = nc.dram_tensor("u1_full", [N, F], F32, addr_space="Shared")
    v_shard = nc.dram_tensor("v_shard", [S, VPAD], F32)
    v_full = nc.dram_tensor("v_full", [N, VPAD], F32, addr_space="Shared")

    rg = [list(range(C))]

    with tile.TileContext(nc) as tc:
        cst = tc.alloc_tile_pool(name="const", bufs=1)
        iota = cst.tile([P, P], F32, name="iota")
        nc.gpsimd.iota(iota[:], pattern=[[1, P]], base=0, channel_multiplier=0,
                       allow_small_or_imprecise_dtypes=True)
        ident = cst.tile([P, P], F32, name="ident")
        make_identity(nc, ident[:])
        w2_sb = cst.tile([F, OUT_CH], F32, name="w2_sb")
        nc.sync.dma_start(out=w2_sb[:], in_=w2_d[:])
        b1_sb = cst.tile([1, F], F32, name="b1_sb")
        nc.sync.dma_start(out=b1_sb[:], in_=b1_d[:])
        b2_sb = cst.tile([1, OUT_CH], F32, name="b2_sb")
        nc.sync.dma_start(out=b2_sb[:], in_=b2_d[:])
        w1_sb = cst.tile([P, KT * F], F32, name="w1_sb")  # kt tile at [:, kt*F:]
        for kt in range(KT):
            kk = min(P, IN_CH - kt * P)
            nc.sync.dma_start(out=w1_sb[:kk, kt * F:(kt + 1) * F],
                              in_=w1_d[kt * P:kt * P + kk, :])
        degc_sb = cst.tile([P, B], F32, name="degc_sb")
        nc.sync.dma_start(out=degc_sb[:], in_=degc_d[:])
        dinv_sb = cst.tile([P, B], F32, name="dinv_sb")
        nc.vector.reciprocal(out=dinv_sb[:], in_=degc_sb[:])
        nc.scalar.activation(out=dinv_sb[:], in_=dinv_sb[:],
                             func=mybir.ActivationFunctionType.Sqrt)
        degr_sb = cst.tile([1, B * P], F32, name="degr_sb")
        nc.sync.dma_start(out=degr_sb[:], in_=degr_d[:])
        sqd_sb = cst.tile([1, B * P], F32, name="sqd_sb")
        nc.scalar.activation(out=sqd_sb[:], in_=degr_sb[:],
                             func=mybir.ActivationFunctionType.Sqrt)
        dl_sb = cst.tile([P, T_total], F32, name="dl_sb")
        nc.sync.dma_start(out=dl_sb[:], in_=dl_d[:])
        vslab = cst.tile([P, B * VPAD], F32, name="vslab")
        nc.gpsimd.memset(vslab[:], 0.0)
        o2slab = cst.tile([P, B * OUT_CH], F32, name="o2slab")

        # ---- Phase A: u1 = dinv * (X @ W1), via hT windows + PE transpose ----
        with (
            tc.tile_pool(name="pa_sb", bufs=3) as pa,
            tc.tile_pool(name="pa_ps", bufs=2, space="PSUM") as pap,
            tc.tile_pool(name="pa_ps_t", bufs=2, space="PSUM") as papt,
        ):
            for w in range(NW):
                cols = min(512, S - w * 512)
                hT_ps = pap.tile([P, 512], F32, tag="hT_ps", name="hT_ps")
                for kt in range(KT):
                    kk = min(P, IN_CH - kt * P)
                    xt_sb = pa.tile([P, 512], F32, tag="xt", name="xt_sb")
                    nc.sync.dma_start(
                        out=xt_sb[:kk, :cols],
                        in_=xt_d[kt * P:kt * P + kk, w * 512:w * 512 + cols])
                    nc.tensor.matmul(out=hT_ps[:, :cols],
                                     lhsT=w1_sb[:kk, kt * F:(kt + 1) * F],
                                     rhs=xt_sb[:kk, :cols],
                                     start=(kt == 0), stop=(kt == KT - 1))
                hT_sb = pa.tile([P, 512], F32, tag="hT_sb", name="hT_sb")
                nc.vector.tensor_copy(out=hT_sb[:, :cols], in_=hT_ps[:, :cols])
                for j in range(math.ceil(cols / P)):
                    cc = min(P, cols - j * P)
                    b = w * 4 + j
                    tps = papt.tile([P, P], F32, tag="tps", name="tps")
                    nc.tensor.transpose(out=tps[:cc, :], in_=hT_sb[:, j * P:j * P + cc],
                                        identity=ident[:])
                    u_sb = pa.tile([P, F], F32, tag="u_sb", name="u_sb")
                    nc.scalar.activation(out=u_sb[:cc, :], in_=tps[:cc, :],
                                         func=mybir.ActivationFunctionType.Copy,
                                         scale=dinv_sb[:cc, b:b + 1])
                    nc.sync.dma_start(out=u1_shard[b * P:b * P + cc, :], in_=u_sb[:cc, :])

        nc.gpsimd.collective_compute(
            "AllGather", mybir.AluOpType.bypass, replica_groups=rg,
            ins=[u1_shard[:]], outs=[u1_full[:]])
        if debug:
            nc.sync.dma_start(out=dbg_u1[:], in_=u1_full[:])

        # ---- Aggregation layers ----
        def agg_layer(layer):
            tab, elem = (u1_full, F) if layer == 0 else (v_full, VPAD)
            tile_ptr = 0
            with (
                tc.tile_pool(name=f"ag_sb{layer}", bufs=2) as ag,
                tc.tile_pool(name=f"ag_acc{layer}", bufs=BC, space="PSUM") as accp,
                tc.tile_pool(name=f"ag_epT{layer}", bufs=1, space="PSUM") as eppt,
                tc.tile_pool(name=f"ag_epV{layer}", bufs=1, space="PSUM") as eppv,
            ):
                for ch in range(n_chunks):
                    blocks = [b for b in range(ch * BC, min((ch + 1) * BC, B))]
                    accs = {b: accp.tile([P, elem], F32, tag="acc",
                                         name=f"acc_{layer}_{ch}_{b}")
                            for b in blocks}

                    def acc_ap(b):
                        return accs[b][:]

                    started = {b: False for b in blocks}
                    for r in range(n_ranges):
                        sec = []  # block id per tile, stream order
                        for b in blocks:
                            sec += [b] * int(tiles[ch, r, b])
                        base = range_bounds[r]
                        rlen = range_bounds[r + 1] - base
                        pos = 0
                        while pos < len(sec):
                            bt = sec[pos:pos + G]
                            gn = len(bt) * P
                            t0 = tile_ptr + pos
                            idx_sb = ag.tile([P, G * P // 16], I16, tag="idx", name="idx_sb")
                            nc.sync.dma_start(
                                out=idx_sb[:, :gn // 16],
                                in_=idx_d[:, (t0 * P) // 16:(t0 * P + gn) // 16])
                            gat = ag.tile([P, G, elem], F32, tag="gat", name="gat")
                            nc.gpsimd.dma_gather(
                                out_ap=gat[:, :len(bt), :],
                                in_ap=tab[base:base + rlen, :],
                                idxs_ap=idx_sb[:, :gn // 16],
                                num_idxs=gn, num_idxs_reg=gn, elem_size=elem,
                                single_packet=False)
                            st = ag.tile([P, G, P], MM_DT, tag="st", name="st")
                            nc.vector.tensor_tensor(
                                out=st[:, :len(bt), :],
                                in0=iota[:].rearrange("p (a f) -> p a f", a=1)
                                    .to_broadcast([P, len(bt), P]),
                                in1=dl_sb[:, t0:t0 + len(bt)]
                                    .rearrange("p (a f) -> p a f", f=1)
                                    .to_broadcast([P, len(bt), P]),
                                op=mybir.AluOpType.is_equal)
                            for i, b in enumerate(bt):
                                nc.tensor.matmul(
                                    out=acc_ap(b), lhsT=st[:, i, :], rhs=gat[:, i, :],
                                    start=(not started[b]), stop=False,
                                    skip_group_check=True)
                                started[b] = True
                            pos += len(bt)
                        tile_ptr += len(sec)

                    # epilogue per block
                    for b in blocks:
                        bias_rhs = b1_sb if layer == 0 else b2_sb
                        nw = F if layer == 0 else OUT_CH
                        nc.tensor.matmul(
                            out=acc_ap(b)[:, :nw],
                            lhsT=sqd_sb[:, b * P:(b + 1) * P],
                            rhs=bias_rhs[:],
                            start=False, stop=True, skip_group_check=True)
                        cc = min(P, S - b * P)
                        if layer == 0 and debug:
                            dbg_sb = ag.tile([P, F], F32, tag="dbg", name="dbg_sb")
                            nc.vector.tensor_copy(out=dbg_sb[:], in_=acc_ap(b))
                            nc.sync.dma_start(out=dbg_agg1[b * P:(b + 1) * P, :],
                                              in_=dbg_sb[:])
                        if layer == 0:
                            x1_sb = ag.tile([P, F], F32, tag="x1", name="x1_sb")
                            nc.scalar.activation(
                                out=x1_sb[:cc, :], in_=acc_ap(b)[:cc, :],
                                func=mybir.ActivationFunctionType.Relu,
                                scale=dinv_sb[:cc, b:b + 1])
                            u2_sb = ag.tile([P, F], F32, tag="u2", name="u2_sb")
                            nc.vector.tensor_scalar(
                                out=u2_sb[:cc, :], in0=x1_sb[:cc, :],
                                scalar1=dinv_sb[:cc, b:b + 1], scalar2=None,
                                op0=mybir.AluOpType.mult)
                            tps2 = epp.tile([P, P], F32, tag="tps2", name="tps2")
                            nc.tensor.transpose(out=tps2[:, :cc], in_=u2_sb[:cc, :],
                                                identity=ident[:cc, :cc])
                            u2T_sb = ag.tile([P, P], F32, tag="u2T", name="u2T_sb")
                            nc.scalar.copy(out=u2T_sb[:, :cc], in_=tps2[:, :cc])
                            v_ps = epp.tile([P, OUT_CH], F32, tag="v_ps", name="v_ps")
                            nc.tensor.matmul(out=v_ps[:cc, :], lhsT=u2T_sb[:, :cc],
                                             rhs=w2_sb[:], start=True, stop=True)
                            nc.vector.tensor_copy(
                                out=vslab[:cc, b * VPAD:b * VPAD + OUT_CH],
                                in_=v_ps[:cc, :])
                        else:
                            nc.scalar.activation(
                                out=o2slab[:cc, b * OUT_CH:(b + 1) * OUT_CH],
                                in_=acc_ap(b)[:cc, :OUT_CH],
                                func=mybir.ActivationFunctionType.Copy,
                                scale=dinv_sb[:cc, b:b + 1])

        agg_layer(0)
        # vslab -> v_shard  (blocks 0..B-2 full, last partial)
        nfull = (S // P)  # number of full blocks
        nc.sync.dma_start(
            out=v_shard[0:nfull * P, :].rearrange("(t p) f -> p t f", p=P),
            in_=vslab[:].rearrange("p (t f) -> p t f", f=VPAD)[:, :nfull, :])
        rem = S - nfull * P
        if rem:
            nc.sync.dma_start(
                out=v_shard[nfull * P:, :],
                in_=vslab[:rem, nfull * VPAD:(nfull + 1) * VPAD])
        nc.gpsimd.collective_compute(
            "AllGather", mybir.AluOpType.bypass, replica_groups=rg,
            ins=[v_shard[:]], outs=[v_full[:]])
        if debug:
            nc.sync.dma_start(out=dbg_v[:], in_=v_full[:])
        agg_layer(1)
        # o2slab -> out  [B*P, OUT_CH]
        nc.sync.dma_start(
            out=out_d[:].rearrange("(t p) f -> p t f", p=P),
            in_=o2slab[:].rearrange("p (t f) -> p t f", f=OUT_CH))
        cst.__exit__(None, None, None)

    nc.finalize()
    return nc


def make_inputs(plan, N, C, features, W1, b1, W2, b2):
    """Per-core input dicts (host-side sharding / transposes)."""
    S, B = plan["S"], plan["B"]
    deg = plan["deg"]
    ins = []
    for c in range(C):
        sl = slice(c * S, (c + 1) * S)
        deg_pad = np.ones(B * P, dtype=np.float32)
        deg_pad[:S] = deg[sl]
        ins.append({
            "xt": np.ascontiguousarray(features[sl].T),
            "w1": np.ascontiguousarray(W1),
            "w2": np.ascontiguousarray(W2),
            "b1r": np.ascontiguousarray(b1[None, :]),
            "b2r": np.ascontiguousarray(b2[None, :]),
            "degc": np.ascontiguousarray(deg_pad.reshape(B, P).T),
            "degr": deg_pad[None, :],
            "idx": plan["idx_streams"][c],
            "dl": plan["dl_streams"][c],
        })
    return ins


# ---------------------------------------------------------------------------
# Harness entry point: full inputs in, full output out.
# ---------------------------------------------------------------------------
N_NODES = 70000
N_CORES = 8
IN_CH_, FEAT_, OUT_ = 1044, 128, 3
RANGE_CAP = 23334        # int16 gather-index limit per table slice
BLOCKS_PER_CHUNK = 6     # live PSUM accumulators per chunk
GATHER_TILES = 32        # 128-edge tiles per dma_gather


def kernel(features, edges, edges2, edge_features, W1, b1, W2, b2):
    """2-layer GCN (PyG GCNConv x2, eval mode). edges2/edge_features unused
    by the module. Returns [70000, 3] float32."""
    from concourse.bass_utils import run_bass_kernel_spmd

    features = np.ascontiguousarray(np.asarray(features, dtype=np.float32))
    W1 = np.asarray(W1, dtype=np.float32)
    b1 = np.asarray(b1, dtype=np.float32)
    W2 = np.asarray(W2, dtype=np.float32)
    b2 = np.asarray(b2, dtype=np.float32)
    e = np.asarray(edges)
    src = e[0].astype(np.int64)
    dst = e[1].astype(np.int64)

    plan = plan_graph(N_NODES, N_CORES, src, dst, range_cap=RANGE_CAP,
                      blocks_per_chunk=BLOCKS_PER_CHUNK)
    nc = build_nc(plan, N_NODES, N_CORES, IN_CH_, FEAT_, OUT_, G=GATHER_TILES)
    ins = make_inputs(plan, N_NODES, N_CORES, features, W1, b1, W2, b2)
    res = run_bass_kernel_spmd(nc, ins, core_ids=list(range(N_CORES)))
    S = plan["S"]
    out = np.concatenate([res.results[c]["out"][:S] for c in range(N_CORES)],
                         axis=0)
    return out.astype(np.float32)
